# revision 86
# baseline (speedup 1.0000x reference)
"""Trainium2 Bass kernel for nn_AttentionSHA (dense transformer attention block).

Full inputs -> full output. Tensor-parallel over heads across 8 NeuronCores
(core g owns kv-head g and query heads 4g..4g+3; wo row-sharded), host-side
reduce of the 8 partial output projections.

v3 (bf16 + software-pipelined schedule):
  - all matmul operands bf16 (f32 PSUM accumulation): halves HBM traffic and
    SBUF footprint vs fp32r; x is kept resident in SBUF (16 DMAs of 4KB/
    partition) so the lead phase computes only K, V and Q0.
  - the Q1..Q3 projections are drip-fed through the attention slots of the
    preceding heads, giving the PE independent work that hides the ACT exp
    stream (9.1us/head) and the DVE normalize chain.
  - softmax denominator z is folded into the PV matmul as a 129th all-ones
    column of V: PV runs "flipped" (stationary = expm s-chunk [t,s], moving
    = [V|1] [t,129]) so out[s,e] and z[s] land with s on partitions, where a
    per-partition reciprocal scale normalizes for free. att is transposed
    back [s,e]->[e,s] on the PE (128 cols each) for the wo projection.
    PSUM allows only one open accumulation group per bank, so the 8 s-chunks
    run as 4 sequential waves of 2 full-bank tiles.
  - during the last head's attention, the first wo output tiles pre-
    accumulate heads 0..2 (closed in phase C), filling the tail bubble.

Math notes (validated against the reference in fp64/fp32 numpy):
  - The reference adds a 0/1 causal mask *before* softmax (no -inf masking)
    and runs softmax over the full MAXSEQ=2048 cache axis where positions
    >= S hold zero k/v. Softmax without max-subtraction is exact here
    (scores are in [-17, 18]), so:
      out = sum_t exp(sc_t)*m_t*v_t / (sum_t exp(sc_t)*m_t + 1024)
    with m_t = e if visible else 1, and +1024 = (MAXSEQ - S) zero-score
    tail. The e-factor for fully-visible regions folds into the Exp bias
    (exp(x + 1) = e*exp(x)); only the 128x128 diagonal blocks need a mask
    multiply.
  - RoPE is applied via host-permuted weight rows (even channels then odd),
    a partition-half swap, and two multiply-adds against [cos;cos] /
    [-sin;sin].
"""
import numpy as np
from collections import deque
from contextlib import ExitStack

S = 1024
D = 4096
NH = 32
NKV = 8
HD = 128
NREP = NH // NKV          # 4
MAXSEQ = 2048
NCORES = 8
DT = D // 128             # 32 d-tiles
TT = S // 128             # 8 t-tiles
NPRE = 2                  # wo tiles pre-accumulated during the last head

_CACHE = {}


def _build_nc(phases=4, repeat=1):
    import concourse.bacc as bacc
    import concourse.mybir as mybir
    import concourse.tile as tile

    f32 = mybir.dt.float32
    bf16 = mybir.dt.bfloat16
    Exp = mybir.ActivationFunctionType.Exp
    Copy = mybir.ActivationFunctionType.Copy
    mult = mybir.AluOpType.mult
    add = mybir.AluOpType.add

    nc = bacc.Bacc("TRN2", target_bir_lowering=False, debug=False,
                   num_devices=NCORES)

    # x host-packed partition-major: col d*1024 + sh*512 + s
    xp = nc.dram_tensor("xp", [128, DT * S], bf16, kind="ExternalInput")
    wq_t = nc.dram_tensor("wq_t", [NREP, 128, DT * HD], bf16, kind="ExternalInput")
    wk_t = nc.dram_tensor("wk_t", [128, DT * HD], bf16, kind="ExternalInput")
    wv_t = nc.dram_tensor("wv_t", [128, DT * HD], bf16, kind="ExternalInput")
    wo_t = nc.dram_tensor("wo_t", [NREP * HD, D], bf16, kind="ExternalInput")
    f16 = mybir.dt.float16
    cc_d = nc.dram_tensor("cc", [HD, S], f16, kind="ExternalInput")
    ns_d = nc.dram_tensor("ns", [HD, S], f16, kind="ExternalInput")
    emaskd_d = nc.dram_tensor("emaskd", [128, 128], bf16, kind="ExternalInput")
    ident_d = nc.dram_tensor("ident", [128, 128], bf16, kind="ExternalInput")
    outT = nc.dram_tensor("outT", [D, S], bf16, kind="ExternalOutput")

    with tile.TileContext(nc) as tc, ExitStack() as ctx:
        const = ctx.enter_context(tc.tile_pool(name="const", bufs=1))
        wts = ctx.enter_context(tc.tile_pool(name="wts", bufs=6))
        xpool = ctx.enter_context(tc.tile_pool(name="xpool", bufs=1))
        rpool = ctx.enter_context(tc.tile_pool(name="rpool", bufs=4))
        qkv = ctx.enter_context(tc.tile_pool(name="qkv", bufs=1))
        hs = ctx.enter_context(tc.tile_pool(name="hs", bufs=5))
        epool = ctx.enter_context(tc.tile_pool(name="epool", bufs=14))
        apool = ctx.enter_context(tc.tile_pool(name="apool", bufs=6))
        zpool = ctx.enter_context(tc.tile_pool(name="zpool", bufs=8))
        opool = ctx.enter_context(tc.tile_pool(name="opool", bufs=5))
        ps = ctx.enter_context(tc.tile_pool(name="ps", bufs=8, space="PSUM"))

        def _body():
            cc_sb = const.tile([128, S], f16)
            ns_sb = const.tile([128, S], f16)
            ident_sb = const.tile([128, 128], bf16)
            emaskd_sb = const.tile([128, 128], bf16)

            wq_sb = [wts.tile([128, D], bf16, name=f"wq_sb{h}", tag="w16")
                     for h in range(NREP)]
            wk_sb = wts.tile([128, D], bf16, tag="w16")
            wv_sb = wts.tile([128, D], bf16, tag="w16")

            # ---- resident x: one big tile, DMAd in 16 slices ----
            xres = xpool.tile([128, DT * S], bf16, name="xres")

            def xsl(d, sh):
                return xres[:, 1024 * d + 512 * sh:1024 * d + 512 * sh + 512]

            # issue order matters: the first K matmul needs wk chunk 0 and
            # x slice 0; V trails K by 4 d-steps, Q0 by 8 (see stagger below)
            nc.sync.dma_start(wk_sb[:, 0:256], wk_t[:, 0:256])
            nc.sync.dma_start(xres[:, 0:1024], xp[:, 0:1024])
            nc.sync.dma_start(wk_sb[:, 256:512], wk_t[:, 256:512])
            nc.sync.dma_start(xres[:, 1024:2048], xp[:, 1024:2048])
            nc.sync.dma_start(wv_sb[:, 0:512], wv_t[:, 0:512])
            nc.sync.dma_start(xres[:, 2048:3072], xp[:, 2048:3072])
            nc.sync.dma_start(wq_sb[0][:, 0:512], wq_t[0][:, 0:512])
            nc.sync.dma_start(xres[:, 3072:4096], xp[:, 3072:4096])
            nc.sync.dma_start(wk_sb[:, 512:1024], wk_t[:, 512:1024])
            nc.sync.dma_start(wv_sb[:, 512:1024], wv_t[:, 512:1024])
            nc.sync.dma_start(wq_sb[0][:, 512:1024], wq_t[0][:, 512:1024])

            if phases < 1:
                nc.sync.dma_start(outT[0:128, :], xres[:, 0:1024])
                return

            q_rot = [hs.tile([128, S], bf16, name=f"q_rot{h}", tag="hs")
                     for h in range(NREP)]
            k_rot = hs.tile([128, S], bf16, tag="hs")
            v_te = [qkv.tile([128, 132], bf16, name=f"v_te{t}") for t in range(TT)]
            for t in range(TT):
                nc.vector.memset(v_te[t][:, 128:129], 1.0)

            # RoPE: dest = psum*[cos;cos] + swap(psum)*[-sin;sin]
            def rope(psum, dest, s0, fast=False):
                sw = rpool.tile([128, 512], f32, name="sw")
                if fast:
                    nc.vector.tensor_copy(sw[0:64, :], psum[64:128, :])
                else:
                    nc.scalar.copy(sw[0:64, :], psum[64:128, :])
                nc.scalar.copy(sw[64:128, :], psum[0:64, :])
                t1 = rpool.tile([128, 512], f32, name="t1")
                nc.vector.tensor_tensor(t1[:], psum[:], cc_sb[:, s0:s0 + 512], op=mult)
                t2 = rpool.tile([128, 512], f32, name="t2")
                nc.gpsimd.tensor_tensor(t2[:], sw[:], ns_sb[:, s0:s0 + 512], op=mult)
                nc.vector.tensor_tensor(dest, t1[:], t2[:], op=add)

            # ---- phase A0: sh0 of K and V, both halves of Q0 ----
            # (sh1 of K/V runs in phase A1, interleaved with head 0's first
            # score/exp t-steps so the ACT exp stream gets a long head start)
            k_ps = [ps.tile([128, 512], f32, tag="ps", name=f"k_ps{sh}")
                    for sh in range(2)]
            q0_ps = [ps.tile([128, 512], f32, tag="ps", name=f"q0_ps{sh}")
                     for sh in range(2)]
            v0_ch = [ps.tile([128, 128], f32, tag="ps", name=f"v0c{c}")
                     for c in range(4)]

            def v_flip_mm(ch, d, start, stop):
                # out[s,e] for s-chunk ch: stationary = x slice, moving = wv
                nc.tensor.matmul(v0_ch[ch % 4][:],
                                 xsl(d, (ch // 4)) [:, 128 * (ch % 4):128 * (ch % 4 + 1)],
                                 wv_sb[:, 128 * d:128 * (d + 1)],
                                 start=start, stop=stop)
            for d in range(DT):
                if d == 0:
                    nc.sync.dma_start(xres[:, 4096:6144], xp[:, 4096:6144])
                if d % 2 == 0 and d // 2 + 3 < 16:
                    i = d // 2 + 3
                    nc.sync.dma_start(xres[:, 2048 * i:2048 * (i + 1)],
                                      xp[:, 2048 * i:2048 * (i + 1)])
                if d % 4 == 0 and d < 24:
                    c0, c1 = 1024 + 512 * (d // 4), 1024 + 512 * (d // 4 + 1)
                    nc.sync.dma_start(wk_sb[:, c0:c1], wk_t[:, c0:c1])
                    nc.sync.dma_start(wv_sb[:, c0:c1], wv_t[:, c0:c1])
                    nc.sync.dma_start(wq_sb[0][:, c0:c1], wq_t[0][:, c0:c1])
                # stagger: V trails K by 4 d-steps, Q0 by 8, so the PE never
                # waits on the wv/wq0 DMAs behind wk/x in the queue
                nc.tensor.matmul(k_ps[0][:], wk_sb[:, 128 * d:128 * (d + 1)],
                                 xsl(d, 0), start=(d == 0), stop=(d == DT - 1))
                if d >= 4:
                    dv = d - 4
                    for cl in range(4):
                        v_flip_mm(cl, dv, dv == 0, dv == DT - 1)
                if d >= 8:
                    dq = d - 8
                    for sh in range(2):
                        nc.tensor.matmul(q0_ps[sh][:], wq_sb[0][:, 128 * dq:128 * (dq + 1)],
                                         xsl(dq, sh), start=(dq == 0), stop=False)
            for dv in range(DT - 4, DT):
                for cl in range(4):
                    v_flip_mm(cl, dv, False, dv == DT - 1)
            for dq in range(DT - 8, DT):
                for sh in range(2):
                    nc.tensor.matmul(q0_ps[sh][:], wq_sb[0][:, 128 * dq:128 * (dq + 1)],
                                     xsl(dq, sh), start=False, stop=(dq == DT - 1))

            rope(k_ps[0], k_rot[:, 0:512], 0, fast=True)
            rope(q0_ps[0], q_rot[0][:, 0:512], 0)
            rope(q0_ps[1], q_rot[0][:, 512:1024], 512)
            for cl in range(4):
                if cl % 2 == 0:
                    nc.scalar.copy(v_te[cl][:, 0:128], v0_ch[cl][:])
                else:
                    nc.vector.tensor_copy(v_te[cl][:, 0:128], v0_ch[cl][:])

            # ---- phase A1: sh1 of K and V (emitted in drippable steps) ----
            # wq1..3 stream here, where the DMA queue is otherwise idle
            v1_ch = {}

            def v1_mm(c, d, start, stop):
                nc.tensor.matmul(v1_ch[c][:],
                                 xsl(d, 1)[:, 128 * (c - 4):128 * (c - 3)],
                                 wv_sb[:, 128 * d:128 * (d + 1)],
                                 start=start, stop=stop)

            def a1_step(d):
                if d == 0:
                    nc.sync.dma_start(cc_sb[:], cc_d[:])
                    nc.sync.dma_start(ns_sb[:], ns_d[:])
                if d == 2:
                    nc.sync.dma_start(ident_sb[:], ident_d[:])
                    nc.sync.dma_start(emaskd_sb[:], emaskd_d[:])
                if d % 4 == 0 and d < 24:
                    g, half = 1 + d // 8, (d // 4) % 2
                    nc.sync.dma_start(wq_sb[g][:, 2048 * half:2048 * (half + 1)],
                                      wq_t[g][:, 2048 * half:2048 * (half + 1)])
                nc.tensor.matmul(k_ps[1][:], wk_sb[:, 128 * d:128 * (d + 1)],
                                 xsl(d, 1), start=(d == 0), stop=(d == DT - 1))
                if d >= 4:
                    dv = d - 4
                    if dv == 0:
                        for c in (4, 5):
                            v1_ch[c] = ps.tile([128, 128], f32, tag="ps",
                                               name=f"v1c{c}")
                    for c in (4, 5):
                        v1_mm(c, dv, dv == 0, dv == DT - 1)

            def a1_tail():
                for dv in range(DT - 4, DT):
                    for c in (4, 5):
                        v1_mm(c, dv, False, dv == DT - 1)
                rope(k_ps[1], k_rot[:, 512:1024], 512, fast=True)
                nc.scalar.copy(v_te[4][:, 0:128], v1_ch[4][:])
                nc.vector.tensor_copy(v_te[5][:, 0:128], v1_ch[5][:])
                for c in (6, 7):
                    v1_ch[c] = ps.tile([128, 128], f32, tag="ps", name=f"v1c{c}")
                for d in range(DT):
                    for c in (6, 7):
                        v1_mm(c, d, d == 0, d == DT - 1)
                nc.scalar.copy(v_te[6][:, 0:128], v1_ch[6][:])
                nc.vector.tensor_copy(v_te[7][:, 0:128], v1_ch[7][:])

            if phases < 2:
                for d in range(DT):
                    a1_step(d)
                a1_tail()
                nc.sync.dma_start(outT[0:128, :], k_rot[:])
                return

            # ---- drip-fed Q1..Q3 projections ----
            qg_ps = {}
            jobs = deque()
            roped = {0}
            for g in range(1, NREP):
                for d in range(DT):
                    jobs.append(('q', g, d))
                jobs.append(('rope', g))

            def drip(n, maxg=NREP):
                while n > 0 and jobs and jobs[0][1] <= maxg:
                    job = jobs.popleft()
                    if job[0] == 'q':
                        _, g, d = job
                        if g not in qg_ps:
                            qg_ps[g] = [ps.tile([128, 512], f32, tag="ps",
                                                name=f"q{g}_ps{sh}")
                                        for sh in range(2)]
                        dl = 128 * d
                        for sh in range(2):
                            nc.tensor.matmul(qg_ps[g][sh][:],
                                             wq_sb[g][:, dl:dl + 128],
                                             xsl(d, sh),
                                             start=(d == 0), stop=(d == DT - 1))
                        n -= 2
                    else:
                        _, g = job
                        rope(qg_ps[g][0], q_rot[g][:, 0:512], 0, fast=True)
                        rope(qg_ps[g][1], q_rot[g][:, 512:1024], 512, fast=True)
                        roped.add(g)

            # ---- attention per head, Q/wo work dripped between waves ----
            att_t = []
            inv_sqrt_hd = float(1.0 / np.sqrt(HD))
            wo_sb = []
            WAVES = [[0, 1], [2, 3], [4, 5], [6, 7]]
            pre_ops = []          # phase-C continuation of pre-accumulated wo
            deferred = []         # (a, c, oz) last-wave finish, done next slot

            def norm_chunk(oz, act=False):
                z_sb = zpool.tile([128, 1], f32, name="z_sb")
                nc.vector.tensor_scalar_add(z_sb[:], oz[:, 128:129],
                                            float(MAXSEQ - S))
                rz = zpool.tile([128, 1], f32, name="rz")
                nc.vector.reciprocal(rz[:], z_sb[:])
                att_n = apool.tile([128, 128], bf16, name="att_n")
                if act:
                    nc.scalar.activation(att_n[:], oz[:, 0:128], Copy,
                                         scale=rz[:])
                else:
                    nc.vector.tensor_scalar_mul(att_n[:], oz[:, 0:128], rz[:])
                return att_n

            def tr_chunk(a, c, att_n, dve=False):
                tr = ps.tile([128, 128], bf16, tag="ps", name="tr")
                nc.tensor.transpose(tr[:], att_n[:], ident_sb[:])
                if not dve and c % 2 == 0:
                    nc.scalar.copy(a[:, 128 * c:128 * (c + 1)], tr[:])
                else:
                    nc.vector.tensor_copy(a[:, 128 * c:128 * (c + 1)], tr[:])

            def make_sc_exp(h):
                def emit_sc_exp(t):
                    dlo, dhi = 128 * t, 128 * (t + 1)
                    expm = epool.tile([128, S], bf16, name="expm")
                    for c in range(2):
                        sc = ps.tile([128, 512], f32, tag="ps", name="sc")
                        nc.tensor.matmul(sc[:], k_rot[:, dlo:dhi],
                                         q_rot[h][:, 512 * c:512 * (c + 1)],
                                         start=True, stop=True)
                        lo, hi = 512 * c, 512 * (c + 1)
                        if dlo >= hi:
                            nc.scalar.activation(expm[:, lo:hi], sc[:], Exp,
                                                 scale=inv_sqrt_hd)
                        elif dhi <= lo:
                            nc.scalar.activation(expm[:, lo:hi], sc[:], Exp,
                                                 scale=inv_sqrt_hd, bias=1.0)
                        else:
                            nc.scalar.activation(expm[:, lo:hi], sc[:], Exp,
                                                 scale=inv_sqrt_hd)
                            nc.gpsimd.tensor_tensor(
                                expm[:, dlo:dhi], expm[:, dlo:dhi],
                                emaskd_sb[:], op=mult)
                            if dhi < hi:
                                nc.gpsimd.tensor_scalar_mul(
                                    expm[:, dhi:hi], expm[:, dhi:hi],
                                    float(np.e))
                    return expm
                return emit_sc_exp

            for h in range(NREP if phases >= 3 else 0):
                if h == 2 and phases >= 4:
                    for g in range(NREP):
                        w = wts.tile([128, D], bf16, name=f"wo_sb{g}", tag="w16")
                        nc.sync.dma_start(w[:], wo_t[128 * g:128 * (g + 1), :])
                        wo_sb.append(w)

                emit_sc_exp = make_sc_exp(h)
                a = hs.tile([128, S], bf16, name=f"att_t{h}", tag="hs")
                expms = []
                if h == 0:
                    # slot h0 interleaves with phase A1: the first 4 score/
                    # exp t-steps run between A1 d-step groups, giving the
                    # ACT exp stream a ~14us head start under PE cover
                    for d in range(0, 8):
                        a1_step(d)
                    pend = [emit_sc_exp(0), emit_sc_exp(1)]
                    oz_w = [ps.tile([128, 512], f32, tag="ps", name=f"oz0_{c}")
                            for c in WAVES[0]]
                    for t in range(4):
                        for d in range(8 + 6 * t, min(8 + 6 * (t + 1), DT)):
                            a1_step(d)
                        if t < 2:
                            pend.append(emit_sc_exp(t + 2))
                        expm_t = pend.pop(0)
                        expms.append(expm_t)
                        for gi, c in enumerate(WAVES[0]):
                            nc.tensor.matmul(oz_w[gi][:, 0:129],
                                             expm_t[:, 128 * c:128 * (c + 1)],
                                             v_te[t][:, 0:129],
                                             start=(t == 0), stop=False)
                    a1_tail()
                    drip(12)
                    pend = [emit_sc_exp(4), emit_sc_exp(5)]
                    for t in range(4, TT):
                        if t + 2 < TT:
                            pend.append(emit_sc_exp(t + 2))
                        expm_t = pend.pop(0)
                        expms.append(expm_t)
                        drip(4, maxg=1)
                        for gi, c in enumerate(WAVES[0]):
                            nc.tensor.matmul(oz_w[gi][:, 0:129],
                                             expm_t[:, 128 * c:128 * (c + 1)],
                                             v_te[t][:, 0:129],
                                             start=False, stop=(t == TT - 1))
                else:
                    # previous head's deferred last wave: norms (DVE) now,
                    # transposes two t-periods later when the chain drains
                    dn = [(da, c, norm_chunk(oz)) for da, c, oz in deferred]
                    deferred.clear()
                    pend = [emit_sc_exp(0), emit_sc_exp(1)]
                    oz_w = [ps.tile([128, 512], f32, tag="ps", name=f"oz{h}_{c}")
                            for c in WAVES[0]]
                    for t in range(TT):
                        if t + 2 < TT:
                            pend.append(emit_sc_exp(t + 2))
                        expm_t = pend.pop(0)
                        expms.append(expm_t)
                        # drip before the PV matmuls: PV(t) waits on exp(t),
                        # and the PE queue is in-order — independent work must
                        # come first or it stalls behind the wait
                        drip(8 if t < 4 else 4, maxg=h + 1)
                        if t == 2:
                            for da, c, att_n in dn:
                                tr_chunk(da, c, att_n, dve=True)
                            dn = []
                        if h == NREP - 1 and phases >= 4 and t in (3, 6):
                            # fill the undrippable last slot: pre-accumulate
                            # heads 0..2 of the first wo tiles (closed in C)
                            do = 0 if t == 3 else 1
                            op_ps = [ps.tile([128, 512], f32, tag="ps",
                                             name=f"opp{do}_{c}") for c in range(2)]
                            for g in range(NREP - 1):
                                for c in range(2):
                                    nc.tensor.matmul(
                                        op_ps[c][:],
                                        wo_sb[g][:, 128 * do:128 * (do + 1)],
                                        att_t[g][:, 512 * c:512 * (c + 1)],
                                        start=(g == 0), stop=False)
                            pre_ops.append((do, op_ps))
                        for gi, c in enumerate(WAVES[0]):
                            nc.tensor.matmul(oz_w[gi][:, 0:129],
                                             expm_t[:, 128 * c:128 * (c + 1)],
                                             v_te[t][:, 0:129],
                                             start=(t == 0), stop=(t == TT - 1))

                while h + 1 < NREP and (h + 1) not in roped:
                    drip(8)
                prev = list(zip(WAVES[0], oz_w))
                for w in range(1, 5):
                    if w < 4:
                        oz_w = [ps.tile([128, 512], f32, tag="ps",
                                        name=f"oz{h}_{c}") for c in WAVES[w]]
                        for gi, c in enumerate(WAVES[w]):
                            for t in range(TT):
                                nc.tensor.matmul(oz_w[gi][:, 0:129],
                                                 expms[t][:, 128 * c:128 * (c + 1)],
                                                 v_te[t][:, 0:129],
                                                 start=(t == 0), stop=(t == TT - 1))
                    else:
                        # defer the last wave's finish into the next slot
                        deferred.extend((a, c, oz) for c, oz in prev)
                        break
                    normed = [(c, norm_chunk(oz, act=True)) for c, oz in prev]
                    drip(8, maxg=h + 2)
                    for c, att_n in normed:
                        tr_chunk(a, c, att_n)
                    prev = list(zip(WAVES[w], oz_w))
                att_t.append(a)

            if phases == 3:
                for h in range(NREP):
                    nc.sync.dma_start(outT[128 * h:128 * (h + 1), :], att_t[h][:])
                return

            # ---- phase C: output projection ----
            # last head's deferred wave finishes first (ops below read att_t[3])
            for da, c, att_n in [(da, c, norm_chunk(oz, act=True))
                                 for da, c, oz in deferred]:
                tr_chunk(da, c, att_n)
            deferred.clear()

            # copy of half c overlaps the matmuls of the next half; one DMA
            # per do-tile (per-half DMAs thrash the staging-slot recycling)
            for do, op_ps in pre_ops:
                out_sb = opool.tile([128, S], bf16, name="out_sb")
                for c in range(2):
                    nc.tensor.matmul(op_ps[c][:],
                                     wo_sb[NREP - 1][:, 128 * do:128 * (do + 1)],
                                     att_t[NREP - 1][:, 512 * c:512 * (c + 1)],
                                     start=False, stop=True)
                    if c == 0:
                        nc.vector.tensor_copy(out_sb[:, 0:512], op_ps[0][:])
                    else:
                        nc.scalar.copy(out_sb[:, 512:1024], op_ps[1][:])
                nc.sync.dma_start(outT[128 * do:128 * (do + 1), :], out_sb[:])

            for do in range(len(pre_ops), DT if phases >= 4 else 0):
                out_sb = opool.tile([128, S], bf16, name="out_sb")
                for c in range(2):
                    op = ps.tile([128, 512], f32, tag="ps", name=f"op{c}")
                    for g in range(NREP):
                        nc.tensor.matmul(op[:],
                                         wo_sb[g][:, 128 * do:128 * (do + 1)],
                                         att_t[g][:, 512 * c:512 * (c + 1)],
                                         start=(g == 0), stop=(g == NREP - 1))
                    if c == 0:
                        nc.vector.tensor_copy(out_sb[:, 0:512], op[:])
                        if do == DT - 1:
                            # final tile: DMA the first half early so the
                            # kernel tail is only half a tile deep
                            nc.sync.dma_start(outT[128 * do:128 * (do + 1), 0:512],
                                              out_sb[:, 0:512])
                    else:
                        nc.scalar.copy(out_sb[:, 512:1024], op[:])
                if do == DT - 1:
                    nc.sync.dma_start(outT[128 * do:128 * (do + 1), 512:1024],
                                      out_sb[:, 512:1024])
                else:
                    nc.sync.dma_start(outT[128 * do:128 * (do + 1), :], out_sb[:])

        for _rep in range(repeat):
            _body()

    nc.compile()
    return nc


def kernel(**inputs):
    import ml_dtypes
    from concourse.bass_utils import run_bass_kernel_spmd
    bf = ml_dtypes.bfloat16

    x = np.asarray(inputs["x"], np.float32)                 # [1, S, D]
    cos = np.asarray(inputs["freqs_cos"], np.float32)       # [S, 64]
    sin = np.asarray(inputs["freqs_sin"], np.float32)       # [S, 64]
    wq = np.asarray(inputs["wq"], np.float32)               # [NH, HD, D]
    wk = np.asarray(inputs["wk"], np.float32)               # [NKV, HD, D]
    wv = np.asarray(inputs["wv"], np.float32)               # [NKV, HD, D]
    wo = np.asarray(inputs["wo"], np.float32)               # [D, D]
    input_pos = np.asarray(inputs["input_pos"]).astype(np.int64)  # [S]

    if "nc" not in _CACHE:
        _CACHE["nc"] = _build_nc()
    nc = _CACHE["nc"]

    perm = np.concatenate([np.arange(0, HD, 2), np.arange(1, HD, 2)])
    # xp: [128, DT*S], col d*1024 + sh*512 + s  (partition = d_within_tile)
    xT = x[0].T                                             # [D, S]
    xp = np.ascontiguousarray(
        xT.reshape(DT, 128, S).transpose(1, 0, 2).reshape(128, DT * S)).astype(bf)
    cc = np.ascontiguousarray(
        np.concatenate([cos.T, cos.T], 0)).astype(np.float16)   # [128, S]
    ns = np.ascontiguousarray(
        np.concatenate([-sin.T, sin.T], 0)).astype(np.float16)  # [128, S]
    # with input_pos an arange, every 128x128 diagonal block has the same
    # visibility pattern — one tile serves all t
    p = input_pos[0:128]
    emaskd = np.where(p[:, None] <= p[None, :], np.float32(np.e),
                      np.float32(1.0)).astype(bf)
    ident = np.eye(128, dtype=np.float32).astype(bf)

    def pmajor(wT):
        return np.ascontiguousarray(
            wT.reshape(DT, 128, HD).transpose(1, 0, 2).reshape(128, DT * HD))

    in_maps = []
    for g in range(NCORES):
        wq_g = wq[NREP * g:NREP * (g + 1)][:, perm, :]       # [4, 128, D]
        in_maps.append({
            "xp": xp,
            "wq_t": np.stack([pmajor(wq_g[j].T) for j in range(NREP)]).astype(bf),
            "wk_t": pmajor(wk[g][perm].T).astype(bf),        # [128, DT*128]
            "wv_t": pmajor(wv[g].T).astype(bf),              # [128, DT*128]
            "wo_t": np.ascontiguousarray(
                wo[:, NREP * HD * g:NREP * HD * (g + 1)].T).astype(bf),  # [512, D]
            "cc": cc, "ns": ns, "emaskd": emaskd, "ident": ident,
        })

    res = run_bass_kernel_spmd(nc, in_maps, list(range(NCORES)))
    total = np.zeros((D, S), np.float64)
    for g in range(NCORES):
        total += res.results[g]["outT"].astype(np.float64)
    return np.ascontiguousarray(total.T.astype(np.float32)[None])   # [1, S, D]
